# revision 6
# baseline (speedup 1.0000x reference)
"""Bahdanau attention kernel for Trainium2, 8-core data-parallel.

Shapes (hardcoded): features [256,225,1280] f32, hidden [256,256] f32,
W1 [1280,256], b1 [256], W2 [256,256], b2 [256], V [256,1], bV [1].
Output: context [256,1280] f32.

Sharding: batch dim split across 8 cores (32 per core); parameters
replicated. No collectives.

Per-core algorithm (batch shard of 32, processed in pairs):
  - load features[b] natural layout [L,D] (2 L-chunks of <=128 partitions)
  - PE-transpose 128x128 blocks -> featT [D,L] (fp32r, 1.5 cyc/row)
  - scoreT[u,l] = tanh(W1.T @ featT + (W2.T @ hiddenT + b1 + b2)) with the
    proj_h+bias term applied as the per-partition bias of the ScalarE tanh
  - logits = V.T @ scoreT  (batch-pair moving dim 450 >= 256 so fp32r
    matmuls run at 1 cycle/row)
  - attn = exp(logits) / sum(exp(logits))  (no max-subtraction needed:
    |logits| <= sum|V| so exp is safe in fp32; bV dropped: softmax-invariant)
  - context = attnT.T @ features_natural  (fp32r, N=512 chunks)
"""

import numpy as np

import concourse.bass as bass
import concourse.bacc as bacc
import concourse.tile as tile
import concourse.mybir as mybir
from concourse import masks
from concourse.bass_utils import run_bass_kernel_spmd

B, L, D, H, U = 256, 225, 1280, 256, 256
NCORES = 8
BS = B // NCORES          # 32 batch items per core
L0, L1 = 128, L - 128     # 128 + 97
DK = D // 128             # 10 d-tiles
F32 = mybir.dt.float32
F32R = mybir.dt.float32r
AF = mybir.ActivationFunctionType


def r(ap):
    """View an fp32 AP as float32r (same bits, fast PE path)."""
    return ap.bitcast(F32R)


def build_kernel():
    nc = bacc.Bacc("TRN2", target_bir_lowering=False, debug=False, num_devices=NCORES)

    feat = nc.dram_tensor("features", [BS, L, D], F32R, kind="ExternalInput").ap()
    hid = nc.dram_tensor("hidden", [BS, H], F32, kind="ExternalInput").ap()
    w1 = nc.dram_tensor("W1", [D, U], F32R, kind="ExternalInput").ap()
    b1 = nc.dram_tensor("b1", [U], F32, kind="ExternalInput").ap()
    w2 = nc.dram_tensor("W2", [H, U], F32R, kind="ExternalInput").ap()
    b2 = nc.dram_tensor("b2", [U], F32, kind="ExternalInput").ap()
    v = nc.dram_tensor("V", [U, 1], F32R, kind="ExternalInput").ap()
    nc.dram_tensor("bV", [1], F32, kind="ExternalInput")  # softmax-invariant
    ctx_out = nc.dram_tensor("context", [BS, D], F32, kind="ExternalOutput").ap()

    with tile.TileContext(nc) as tc:
        body(tc, feat, hid, w1, b1, w2, b2, v, ctx_out)
    nc.compile()
    return nc


def body(tc, feat, hid, w1, b1, w2, b2, v, ctx_out):
    nc = tc.nc
    from contextlib import ExitStack

    with ExitStack() as ctx:
        const = ctx.enter_context(tc.tile_pool(name="const", bufs=1))
        fnat_pool = ctx.enter_context(tc.tile_pool(name="fnat", bufs=4))
        featT_pool = ctx.enter_context(tc.tile_pool(name="featT", bufs=2))
        score_pool = ctx.enter_context(tc.tile_pool(name="score", bufs=2))
        small = ctx.enter_context(tc.tile_pool(name="small", bufs=2))
        outst_pool = ctx.enter_context(tc.tile_pool(name="outst", bufs=2))
        # One PSUM pool; bank budget (8 total): trp*2 + scp0 + scp1 + lgp + cxp*2 = 7
        pp = ctx.enter_context(tc.tile_pool(name="pp", bufs=1, space="PSUM"))

        # ---- constants ----
        ident = const.tile([128, 128], F32)
        masks.make_identity(nc, ident[:, :])
        ident_r = const.tile([128, 128], F32R)
        nc.vector.tensor_copy(ident_r[:, :], ident[:, :])

        ones32 = const.tile([1, 32], F32)
        nc.gpsimd.memset(ones32[:, :], 1.0)
        ones32_r = const.tile([1, 32], F32R)
        nc.vector.tensor_copy(ones32_r[:, :], ones32[:, :])

        w1_sb = const.tile([128, DK, U], F32R)  # [d_in_tile, d_tile, u]
        nc.sync.dma_start(w1_sb[:, :, :], w1.rearrange("(k p) u -> p k u", p=128))

        w2_sb = const.tile([128, 2, U], F32R)   # [h_in_tile, h_tile, u]
        nc.sync.dma_start(w2_sb[:, :, :], w2.rearrange("(k p) u -> p k u", p=128))

        v_sb = const.tile([128, 2], F32R)       # [u_in_tile, u_tile]
        nc.sync.dma_start(v_sb[:, :], v.rearrange("(t p) o -> p (t o)", p=128))

        bsum = const.tile([1, U], F32R)         # b1 + b2 (both added pre-tanh)
        b1_sb = const.tile([1, U], F32)
        b2_sb = const.tile([1, U], F32)
        nc.sync.dma_start(b1_sb[:, :], b1[None, :])
        nc.sync.dma_start(b2_sb[:, :], b2[None, :])
        nc.vector.tensor_add(bsum[:, :], b1_sb[:, :], b2_sb[:, :])

        # ---- proj_hT [u, b] = W2.T @ hiddenT + (b1+b2) ----
        hid_nat = const.tile([32, H], F32)
        nc.sync.dma_start(hid_nat[:, :], hid[:, :])

        hidT = const.tile([128, 2, BS], F32R)   # [h_in_tile, h_tile, b]
        if True:
            for hk in range(2):
                hp = pp.tile([128, 256], F32, tag="trp", bufs=2)
                nc.tensor.transpose(
                    hp[:, 0:32], hid_nat[0:32, hk * 128:(hk + 1) * 128],
                    ident[0:32, 0:32])
                nc.vector.tensor_copy(hidT[:, hk, :], hp[:, 0:32])
            projhT = const.tile([128, 2 * BS], F32)  # [u_in_tile, ut*32+b]
            for ut in range(2):
                php = pp.tile([128, 256], F32, tag="trp", bufs=2)
                for hk in range(2):
                    nc.tensor.matmul(
                        php[:, 0:32],
                        lhsT=w2_sb[:, hk, ut * 128:(ut + 1) * 128],
                        rhs=hidT[:, hk, :],
                        start=(hk == 0), stop=False)
                nc.tensor.matmul(
                    php[:, 0:32],
                    lhsT=bsum[0:1, ut * 128:(ut + 1) * 128],
                    rhs=ones32_r[0:1, :].opt(),
                    start=False, stop=True)
                nc.vector.tensor_copy(projhT[:, ut * BS:(ut + 1) * BS], php[:, 0:32])

        # ---- main loop over batch pairs ----
        flat_out = ctx_out.rearrange("b d -> (b d)")
        out_stage = None
        for pi in range(BS // 2):
            if pi % 2 == 0:
                out_stage = outst_pool.tile([1, 4 * D], F32, tag="out_stage")
            fnats = []
            for half in range(2):
                b = 2 * pi + half
                fnatA = fnat_pool.tile([128, D], F32R, tag=f"fnatA{half}")
                fnatB = fnat_pool.tile([128, D], F32R, tag=f"fnatB{half}")
                nc.sync.dma_start(fnatA[:, :], feat[b, 0:128, :])
                nc.sync.dma_start(fnatB[0:L1, :], feat[b, 128:L, :])
                fnats.append((fnatA, fnatB))

            # transpose features of both halves -> featT [d_p, k, pairL]
            featT = featT_pool.tile([128, DK, 2 * L], F32R, tag="featT")
            for half in range(2):
                fnatA, fnatB = fnats[half]
                for k in range(DK):
                    trp = pp.tile([128, 256], F32, tag="trp", bufs=2)
                    nc.tensor.transpose(
                        r(trp[:, 0:128]),
                        fnatA[:, k * 128:(k + 1) * 128],
                        ident_r[:, :])
                    # 98 rows (L1=97 rounded up): fp32r transpose needs even
                    # moving/dst counts; the garbage col lands at trp[:,225],
                    # outside the [0:225] copy below.
                    nc.tensor.transpose(
                        r(trp[:, 128:128 + L1 + 1]),
                        fnatB[0:L1 + 1, k * 128:(k + 1) * 128],
                        ident_r[0:L1 + 1, 0:L1 + 1])
                    nc.vector.tensor_copy(
                        featT[:, k, half * L:(half + 1) * L], trp[:, 0:L])

            # scoreT = tanh(W1.T @ featT + projh_bias)  [u, pairL]
            score_sb = score_pool.tile([128, 2, 2 * L], F32R, tag="score_sb")
            for ut in range(2):
                scp = pp.tile([128, 512], F32, tag=f"scp{ut}", bufs=1)
                for k in range(DK):
                    nc.tensor.matmul(
                        scp[:, 0:2 * L],
                        lhsT=w1_sb[:, k, ut * 128:(ut + 1) * 128],
                        rhs=featT[:, k, :],
                        start=(k == 0), stop=(k == DK - 1))
                for half in range(2):
                    b = 2 * pi + half
                    nc.scalar.activation(
                        score_sb[:, ut, half * L:(half + 1) * L],
                        scp[:, half * L:(half + 1) * L],
                        AF.Tanh,
                        bias=projhT[:, ut * BS + b:ut * BS + b + 1])

            # logits [1, 2L] = V.T @ scoreT
            lgp = pp.tile([1, 512], F32, tag="lgp", bufs=1)
            for ut in range(2):
                nc.tensor.matmul(
                    lgp[0:1, 0:2 * L],
                    lhsT=v_sb[:, ut:ut + 1],
                    rhs=score_sb[:, ut, :],
                    start=(ut == 0), stop=(ut == 1))

            # exp + per-half sums; attn = expl / sum
            expl = small.tile([1, 2 * L], F32, tag="expl")
            esum = small.tile([1, 2], F32, tag="esum")
            for half in range(2):
                nc.scalar.activation(
                    expl[0:1, half * L:(half + 1) * L],
                    lgp[0:1, half * L:(half + 1) * L],
                    AF.Exp,
                    accum_out=esum[0:1, half:half + 1])
            rsum = small.tile([1, 2], F32, tag="rsum")
            nc.vector.reciprocal(rsum[:, :], esum[:, :])
            attn = small.tile([1, 2 * L], F32, tag="attn")
            for half in range(2):
                nc.vector.tensor_scalar_mul(
                    attn[0:1, half * L:(half + 1) * L],
                    expl[0:1, half * L:(half + 1) * L],
                    rsum[0:1, half:half + 1])

            # attnT columns: [l_p, half*2+chunk]
            atp = pp.tile([128, 256], F32, tag="trp", bufs=2)
            attnT = small.tile([128, 4], F32R, tag="attnT")
            for half in range(2):
                nc.tensor.transpose(
                    atp[0:128, 2 * half:2 * half + 1],
                    attn[0:1, half * L:half * L + 128],
                    ident[0:1, 0:1])
                nc.tensor.transpose(
                    atp[0:L1, 2 * half + 1:2 * half + 2],
                    attn[0:1, half * L + 128:half * L + L],
                    ident[0:1, 0:1])
                nc.vector.tensor_copy(attnT[0:128, 2 * half:2 * half + 1],
                                      atp[0:128, 2 * half:2 * half + 1])
                nc.vector.tensor_copy(attnT[0:L1, 2 * half + 1:2 * half + 2],
                                      atp[0:L1, 2 * half + 1:2 * half + 2])

            # context[b, :] = attnT.T @ feat_natural
            for half in range(2):
                b = 2 * pi + half
                fnatA, fnatB = fnats[half]
                for doff, dw in ((0, 512), (512, 512), (1024, 256)):
                    cxp = pp.tile([1, 512], F32, tag="cxp", bufs=2)
                    nc.tensor.matmul(
                        cxp[0:1, 0:dw],
                        lhsT=attnT[0:128, 2 * half:2 * half + 1],
                        rhs=fnatA[:, doff:doff + dw],
                        start=True, stop=False)
                    nc.tensor.matmul(
                        cxp[0:1, 0:dw],
                        lhsT=attnT[0:L1, 2 * half + 1:2 * half + 2],
                        rhs=fnatB[0:L1, doff:doff + dw],
                        start=False, stop=True)
                    nc.scalar.copy(
                        out_stage[0:1, (b % 4) * D + doff:(b % 4) * D + doff + dw],
                        cxp[0:1, 0:dw])

            if pi % 2 == 1:
                g = pi // 2
                nc.sync.dma_start(
                    flat_out[4 * g * D:(4 * g + 4) * D][None, :],
                    out_stage[0:1, :])


_CACHE = {}


def _get_nc():
    if "nc" not in _CACHE:
        _CACHE["nc"] = build_kernel()
    return _CACHE["nc"]


def _run(inputs, trace=False):
    nc = _get_nc()
    in_maps = []
    for c in range(NCORES):
        sl = slice(c * BS, (c + 1) * BS)
        in_maps.append({
            "features": np.ascontiguousarray(inputs["features"][sl]),
            "hidden": np.ascontiguousarray(inputs["hidden"][sl]),
            "W1": np.ascontiguousarray(inputs["W1"]),
            "b1": np.ascontiguousarray(inputs["b1"]),
            "W2": np.ascontiguousarray(inputs["W2"]),
            "b2": np.ascontiguousarray(inputs["b2"]),
            "V": np.ascontiguousarray(inputs["V"]),
            "bV": np.ascontiguousarray(inputs["bV"]),
        })
    res = run_bass_kernel_spmd(nc, in_maps, core_ids=list(range(NCORES)),
                               trace=trace)
    out = np.concatenate([rr["context"] for rr in res.results], axis=0)
    return out, res


def kernel(**inputs):
    out, _ = _run(inputs, trace=False)
    return out


# revision 7
# speedup vs baseline: 2.1957x; 2.1957x over previous
"""Bahdanau attention kernel for Trainium2, 8-core data-parallel.

Shapes (hardcoded): features [256,225,1280] f32, hidden [256,256] f32,
W1 [1280,256], b1 [256], W2 [256,256], b2 [256], V [256,1], bV [1].
Output: context [256,1280] f32.

Sharding: batch dim split across 8 cores (32 per core); parameters
replicated. No collectives.

Per-core algorithm (batch shard of 32, processed in pairs):
  - load features[b] natural layout [L,D] (2 L-chunks of <=128 partitions)
  - PE-transpose 128x128 blocks -> featT [D,L] (fp32r, 1.5 cyc/row)
  - scoreT[u,l] = tanh(W1.T @ featT + (W2.T @ hiddenT + b1 + b2)) with the
    proj_h+bias term applied as the per-partition bias of the ScalarE tanh
  - logits = V.T @ scoreT  (batch-pair moving dim 450 >= 256 so fp32r
    matmuls run at 1 cycle/row)
  - attn = exp(logits) / sum(exp(logits))  (no max-subtraction needed:
    |logits| <= sum|V| so exp is safe in fp32; bV dropped: softmax-invariant)
  - context = attnT.T @ features_natural  (fp32r, N=512 chunks)
"""

import numpy as np

import concourse.bass as bass
import concourse.bacc as bacc
import concourse.tile as tile
import concourse.mybir as mybir
from concourse import masks
from concourse.bass_utils import run_bass_kernel_spmd

B, L, D, H, U = 256, 225, 1280, 256, 256
NCORES = 8
BS = B // NCORES          # 32 batch items per core
L0, L1 = 128, L - 128     # 128 + 97
DK = D // 128             # 10 d-tiles
F32 = mybir.dt.float32
F32R = mybir.dt.float32r
BF16 = mybir.dt.bfloat16
AF = mybir.ActivationFunctionType


def r(ap):
    """View an fp32 AP as float32r (same bits, fast PE path)."""
    return ap.bitcast(F32R)


def build_kernel():
    nc = bacc.Bacc("TRN2", target_bir_lowering=False, debug=False, num_devices=NCORES)

    feat = nc.dram_tensor("features", [BS, L, D], F32, kind="ExternalInput").ap()
    hid = nc.dram_tensor("hidden", [BS, H], F32, kind="ExternalInput").ap()
    w1 = nc.dram_tensor("W1", [D, U], F32, kind="ExternalInput").ap()
    b1 = nc.dram_tensor("b1", [U], F32, kind="ExternalInput").ap()
    w2 = nc.dram_tensor("W2", [H, U], F32, kind="ExternalInput").ap()
    b2 = nc.dram_tensor("b2", [U], F32, kind="ExternalInput").ap()
    v = nc.dram_tensor("V", [U, 1], F32, kind="ExternalInput").ap()
    nc.dram_tensor("bV", [1], F32, kind="ExternalInput")  # softmax-invariant
    ctx_out = nc.dram_tensor("context", [BS, D], F32, kind="ExternalOutput").ap()

    with tile.TileContext(nc) as tc:
        body(tc, feat, hid, w1, b1, w2, b2, v, ctx_out)
    nc.compile()
    return nc


def body(tc, feat, hid, w1, b1, w2, b2, v, ctx_out):
    nc = tc.nc
    from contextlib import ExitStack

    with ExitStack() as ctx:
        const = ctx.enter_context(tc.tile_pool(name="const", bufs=1))
        fnat_pool = ctx.enter_context(tc.tile_pool(name="fnat", bufs=4))
        featT_pool = ctx.enter_context(tc.tile_pool(name="featT", bufs=2))
        score_pool = ctx.enter_context(tc.tile_pool(name="score", bufs=2))
        small = ctx.enter_context(tc.tile_pool(name="small", bufs=2))
        outst_pool = ctx.enter_context(tc.tile_pool(name="outst", bufs=2))
        # One PSUM pool; bank budget (8 total): trp*2 + scp0 + scp1 + lgp + cxp*2 = 7
        pp = ctx.enter_context(tc.tile_pool(name="pp", bufs=1, space="PSUM"))

        # ---- constants ----
        ident = const.tile([128, 128], F32)
        masks.make_identity(nc, ident[:, :])
        ident_r = const.tile([128, 128], BF16)
        nc.vector.tensor_copy(ident_r[:, :], ident[:, :])

        ones32 = const.tile([1, 32], F32)
        nc.gpsimd.memset(ones32[:, :], 1.0)
        ones32_r = const.tile([1, 32], BF16)
        nc.vector.tensor_copy(ones32_r[:, :], ones32[:, :])

        w1_sb = const.tile([128, DK, U], BF16)  # [d_in_tile, d_tile, u]
        nc.gpsimd.dma_start(w1_sb[:, :, :], w1.rearrange("(k p) u -> p k u", p=128))

        w2_sb = const.tile([128, 2, U], BF16)   # [h_in_tile, h_tile, u]
        nc.gpsimd.dma_start(w2_sb[:, :, :], w2.rearrange("(k p) u -> p k u", p=128))

        v_sb = const.tile([128, 2], BF16)       # [u_in_tile, u_tile]
        nc.gpsimd.dma_start(v_sb[:, :], v.rearrange("(t p) o -> p (t o)", p=128))

        bsum = const.tile([1, U], BF16)         # b1 + b2 (both added pre-tanh)
        b1_sb = const.tile([1, U], F32)
        b2_sb = const.tile([1, U], F32)
        nc.sync.dma_start(b1_sb[:, :], b1[None, :])
        nc.sync.dma_start(b2_sb[:, :], b2[None, :])
        nc.vector.tensor_add(bsum[:, :], b1_sb[:, :], b2_sb[:, :])

        # ---- proj_hT [u, b] = W2.T @ hiddenT + (b1+b2) ----
        hid_nat = const.tile([32, H], F32)
        nc.sync.dma_start(hid_nat[:, :], hid[:, :])

        hidT = const.tile([128, 2, BS], BF16)   # [h_in_tile, h_tile, b]
        if True:
            for hk in range(2):
                hp = pp.tile([128, 256], F32, tag="trp", bufs=2)
                nc.tensor.transpose(
                    hp[:, 0:32], hid_nat[0:32, hk * 128:(hk + 1) * 128],
                    ident[0:32, 0:32])
                nc.vector.tensor_copy(hidT[:, hk, :], hp[:, 0:32])
            projhT = const.tile([128, 2 * BS], F32)  # [u_in_tile, ut*32+b]
            for ut in range(2):
                php = pp.tile([128, 256], F32, tag="trp", bufs=2)
                for hk in range(2):
                    nc.tensor.matmul(
                        php[:, 0:32],
                        lhsT=w2_sb[:, hk, ut * 128:(ut + 1) * 128],
                        rhs=hidT[:, hk, :],
                        start=(hk == 0), stop=False)
                nc.tensor.matmul(
                    php[:, 0:32],
                    lhsT=bsum[0:1, ut * 128:(ut + 1) * 128],
                    rhs=ones32_r[0:1, :].opt(),
                    start=False, stop=True)
                nc.vector.tensor_copy(projhT[:, ut * BS:(ut + 1) * BS], php[:, 0:32])

        # ---- main loop over batch pairs ----
        flat_out = ctx_out.rearrange("b d -> (b d)")
        out_stage = None
        for pi in range(BS // 2):
            if pi % 2 == 0:
                out_stage = outst_pool.tile([1, 4 * D], F32, tag="out_stage")
            fnats = []
            for half in range(2):
                b = 2 * pi + half
                fnatA = fnat_pool.tile([128, D], BF16, tag=f"fnatA{half}")
                fnatB = fnat_pool.tile([128, D], BF16, tag=f"fnatB{half}")
                nc.gpsimd.dma_start(fnatA[:, :], feat[b, 0:128, :])
                nc.gpsimd.dma_start(fnatB[0:L1, :], feat[b, 128:L, :])
                fnats.append((fnatA, fnatB))

            # transpose features of both halves -> featT [d_p, k, pairL]
            featT = featT_pool.tile([128, DK, 2 * L], BF16, tag="featT")
            for half in range(2):
                fnatA, fnatB = fnats[half]
                for k in range(DK):
                    trp = pp.tile([128, 256], BF16, tag="trp", bufs=2)
                    nc.tensor.transpose(
                        trp[:, 0:128],
                        fnatA[:, k * 128:(k + 1) * 128],
                        ident_r[:, :])
                    # 98 rows (L1=97 rounded up): fp32r transpose needs even
                    # moving/dst counts; the garbage col lands at trp[:,225],
                    # outside the [0:225] copy below.
                    nc.tensor.transpose(
                        trp[:, 128:128 + L1 + 1],
                        fnatB[0:L1 + 1, k * 128:(k + 1) * 128],
                        ident_r[0:L1 + 1, 0:L1 + 1])
                    nc.vector.tensor_copy(
                        featT[:, k, half * L:(half + 1) * L], trp[:, 0:L])

            # scoreT = tanh(W1.T @ featT + projh_bias)  [u, pairL]
            score_sb = score_pool.tile([128, 2, 2 * L], BF16, tag="score_sb")
            for ut in range(2):
                scp = pp.tile([128, 512], F32, tag=f"scp{ut}", bufs=1)
                for k in range(DK):
                    nc.tensor.matmul(
                        scp[:, 0:2 * L],
                        lhsT=w1_sb[:, k, ut * 128:(ut + 1) * 128],
                        rhs=featT[:, k, :],
                        start=(k == 0), stop=(k == DK - 1))
                for half in range(2):
                    b = 2 * pi + half
                    nc.scalar.activation(
                        score_sb[:, ut, half * L:(half + 1) * L],
                        scp[:, half * L:(half + 1) * L],
                        AF.Tanh,
                        bias=projhT[:, ut * BS + b:ut * BS + b + 1])

            # logits [1, 2L] = V.T @ scoreT
            lgp = pp.tile([1, 512], F32, tag="lgp", bufs=1)
            for ut in range(2):
                nc.tensor.matmul(
                    lgp[0:1, 0:2 * L],
                    lhsT=v_sb[:, ut:ut + 1],
                    rhs=score_sb[:, ut, :],
                    start=(ut == 0), stop=(ut == 1))

            # exp + per-half sums; attn = expl / sum
            expl = small.tile([1, 2 * L], F32, tag="expl")
            esum = small.tile([1, 2], F32, tag="esum")
            for half in range(2):
                nc.scalar.activation(
                    expl[0:1, half * L:(half + 1) * L],
                    lgp[0:1, half * L:(half + 1) * L],
                    AF.Exp,
                    accum_out=esum[0:1, half:half + 1])
            rsum = small.tile([1, 2], F32, tag="rsum")
            nc.vector.reciprocal(rsum[:, :], esum[:, :])
            attn = small.tile([1, 2 * L], F32, tag="attn")
            for half in range(2):
                nc.vector.tensor_scalar_mul(
                    attn[0:1, half * L:(half + 1) * L],
                    expl[0:1, half * L:(half + 1) * L],
                    rsum[0:1, half:half + 1])

            # attnT columns: [l_p, half*2+chunk]
            atp = pp.tile([128, 256], F32, tag="trp", bufs=2)
            attnT = small.tile([128, 4], BF16, tag="attnT")
            for half in range(2):
                nc.tensor.transpose(
                    atp[0:128, 2 * half:2 * half + 1],
                    attn[0:1, half * L:half * L + 128],
                    ident[0:1, 0:1])
                nc.tensor.transpose(
                    atp[0:L1, 2 * half + 1:2 * half + 2],
                    attn[0:1, half * L + 128:half * L + L],
                    ident[0:1, 0:1])
                nc.vector.tensor_copy(attnT[0:128, 2 * half:2 * half + 1],
                                      atp[0:128, 2 * half:2 * half + 1])
                nc.vector.tensor_copy(attnT[0:L1, 2 * half + 1:2 * half + 2],
                                      atp[0:L1, 2 * half + 1:2 * half + 2])

            # context[b, :] = attnT.T @ feat_natural
            for half in range(2):
                b = 2 * pi + half
                fnatA, fnatB = fnats[half]
                for doff, dw in ((0, 512), (512, 512), (1024, 256)):
                    cxp = pp.tile([1, 512], F32, tag="cxp", bufs=2)
                    nc.tensor.matmul(
                        cxp[0:1, 0:dw],
                        lhsT=attnT[0:128, 2 * half:2 * half + 1],
                        rhs=fnatA[:, doff:doff + dw],
                        start=True, stop=False)
                    nc.tensor.matmul(
                        cxp[0:1, 0:dw],
                        lhsT=attnT[0:L1, 2 * half + 1:2 * half + 2],
                        rhs=fnatB[0:L1, doff:doff + dw],
                        start=False, stop=True)
                    nc.scalar.copy(
                        out_stage[0:1, (b % 4) * D + doff:(b % 4) * D + doff + dw],
                        cxp[0:1, 0:dw])

            if pi % 2 == 1:
                g = pi // 2
                nc.sync.dma_start(
                    flat_out[4 * g * D:(4 * g + 4) * D][None, :],
                    out_stage[0:1, :])


_CACHE = {}


def _get_nc():
    if "nc" not in _CACHE:
        _CACHE["nc"] = build_kernel()
    return _CACHE["nc"]


def _run(inputs, trace=False):
    nc = _get_nc()
    in_maps = []
    for c in range(NCORES):
        sl = slice(c * BS, (c + 1) * BS)
        in_maps.append({
            "features": np.ascontiguousarray(inputs["features"][sl]),
            "hidden": np.ascontiguousarray(inputs["hidden"][sl]),
            "W1": np.ascontiguousarray(inputs["W1"]),
            "b1": np.ascontiguousarray(inputs["b1"]),
            "W2": np.ascontiguousarray(inputs["W2"]),
            "b2": np.ascontiguousarray(inputs["b2"]),
            "V": np.ascontiguousarray(inputs["V"]),
            "bV": np.ascontiguousarray(inputs["bV"]),
        })
    res = run_bass_kernel_spmd(nc, in_maps, core_ids=list(range(NCORES)),
                               trace=trace)
    out = np.concatenate([rr["context"] for rr in res.results], axis=0)
    return out, res


def kernel(**inputs):
    out, _ = _run(inputs, trace=False)
    return out


# revision 8
# speedup vs baseline: 2.8766x; 1.3101x over previous
"""Bahdanau attention kernel for Trainium2, 8-core data-parallel.

Shapes (hardcoded): features [256,225,1280] f32, hidden [256,256] f32,
W1 [1280,256], b1 [256], W2 [256,256], b2 [256], V [256,1], bV [1].
Output: context [256,1280] f32.

Sharding: batch dim split across 8 cores (32 per core); parameters
replicated. No collectives.

Per-core algorithm (batch shard of 32, processed in pairs):
  - load features[b] natural layout [L,D] (2 L-chunks of <=128 partitions)
  - PE-transpose 128x128 blocks -> featT [D,L] (fp32r, 1.5 cyc/row)
  - scoreT[u,l] = tanh(W1.T @ featT + (W2.T @ hiddenT + b1 + b2)) with the
    proj_h+bias term applied as the per-partition bias of the ScalarE tanh
  - logits = V.T @ scoreT  (batch-pair moving dim 450 >= 256 so fp32r
    matmuls run at 1 cycle/row)
  - attn = exp(logits) / sum(exp(logits))  (no max-subtraction needed:
    |logits| <= sum|V| so exp is safe in fp32; bV dropped: softmax-invariant)
  - context = attnT.T @ features_natural  (fp32r, N=512 chunks)
"""

import numpy as np

import concourse.bass as bass
import concourse.bacc as bacc
import concourse.tile as tile
import concourse.mybir as mybir
from concourse import masks
from concourse.bass_utils import run_bass_kernel_spmd

B, L, D, H, U = 256, 225, 1280, 256, 256
NCORES = 8
BS = B // NCORES          # 32 batch items per core
L0, L1 = 128, L - 128     # 128 + 97
DK = D // 128             # 10 d-tiles
F32 = mybir.dt.float32
F32R = mybir.dt.float32r
BF16 = mybir.dt.bfloat16
AF = mybir.ActivationFunctionType


def r(ap):
    """View an fp32 AP as float32r (same bits, fast PE path)."""
    return ap.bitcast(F32R)


def build_kernel():
    nc = bacc.Bacc("TRN2", target_bir_lowering=False, debug=False, num_devices=NCORES)

    feat = nc.dram_tensor("features", [BS, L, D], F32, kind="ExternalInput").ap()
    hid = nc.dram_tensor("hidden", [BS, H], F32, kind="ExternalInput").ap()
    w1 = nc.dram_tensor("W1", [D, U], F32, kind="ExternalInput").ap()
    b1 = nc.dram_tensor("b1", [U], F32, kind="ExternalInput").ap()
    w2 = nc.dram_tensor("W2", [H, U], F32, kind="ExternalInput").ap()
    b2 = nc.dram_tensor("b2", [U], F32, kind="ExternalInput").ap()
    v = nc.dram_tensor("V", [U, 1], F32, kind="ExternalInput").ap()
    nc.dram_tensor("bV", [1], F32, kind="ExternalInput")  # softmax-invariant
    ctx_out = nc.dram_tensor("context", [BS, D], F32, kind="ExternalOutput").ap()

    with tile.TileContext(nc) as tc:
        body(tc, feat, hid, w1, b1, w2, b2, v, ctx_out)
    nc.compile()
    return nc


def body(tc, feat, hid, w1, b1, w2, b2, v, ctx_out):
    nc = tc.nc
    from contextlib import ExitStack

    with ExitStack() as ctx:
        const = ctx.enter_context(tc.tile_pool(name="const", bufs=1))
        fnat_pool = ctx.enter_context(tc.tile_pool(name="fnat", bufs=6))
        featT_pool = ctx.enter_context(tc.tile_pool(name="featT", bufs=3))
        score_pool = ctx.enter_context(tc.tile_pool(name="score", bufs=3))
        small = ctx.enter_context(tc.tile_pool(name="small", bufs=2))
        outst_pool = ctx.enter_context(tc.tile_pool(name="outst", bufs=2))
        # One PSUM pool; bank budget (8 total): trp*2 + scp0 + scp1 + lgp + cxp*2 = 7
        pp = ctx.enter_context(tc.tile_pool(name="pp", bufs=1, space="PSUM"))

        # ---- constants ----
        ident = const.tile([128, 128], F32)
        masks.make_identity(nc, ident[:, :])
        ident_r = const.tile([128, 128], BF16)
        nc.vector.tensor_copy(ident_r[:, :], ident[:, :])

        ones32 = const.tile([1, 32], F32)
        nc.gpsimd.memset(ones32[:, :], 1.0)
        ones32_r = const.tile([1, 32], BF16)
        nc.vector.tensor_copy(ones32_r[:, :], ones32[:, :])

        w1_sb = const.tile([128, DK, U], BF16)  # [d_in_tile, d_tile, u]
        nc.gpsimd.dma_start(w1_sb[:, :, :], w1.rearrange("(k p) u -> p k u", p=128))

        w2_sb = const.tile([128, 2, U], BF16)   # [h_in_tile, h_tile, u]
        nc.gpsimd.dma_start(w2_sb[:, :, :], w2.rearrange("(k p) u -> p k u", p=128))

        v_sb = const.tile([128, 2], BF16)       # [u_in_tile, u_tile]
        nc.gpsimd.dma_start(v_sb[:, :], v.rearrange("(t p) o -> p (t o)", p=128))

        bsum = const.tile([1, U], BF16)         # b1 + b2 (both added pre-tanh)
        b1_sb = const.tile([1, U], F32)
        b2_sb = const.tile([1, U], F32)
        nc.sync.dma_start(b1_sb[:, :], b1[None, :])
        nc.sync.dma_start(b2_sb[:, :], b2[None, :])
        nc.vector.tensor_add(bsum[:, :], b1_sb[:, :], b2_sb[:, :])

        # ---- proj_hT [u, b] = W2.T @ hiddenT + (b1+b2) ----
        hid_nat = const.tile([32, H], F32)
        nc.sync.dma_start(hid_nat[:, :], hid[:, :])

        hidT = const.tile([128, 2, BS], BF16)   # [h_in_tile, h_tile, b]
        if True:
            for hk in range(2):
                hp = pp.tile([128, 256], F32, tag="trp", bufs=3)
                nc.tensor.transpose(
                    hp[:, 0:32], hid_nat[0:32, hk * 128:(hk + 1) * 128],
                    ident[0:32, 0:32])
                nc.vector.tensor_copy(hidT[:, hk, :], hp[:, 0:32])
            projhT = const.tile([128, 2 * BS], F32)  # [u_in_tile, ut*32+b]
            for ut in range(2):
                php = pp.tile([128, 256], F32, tag="trp", bufs=3)
                for hk in range(2):
                    nc.tensor.matmul(
                        php[:, 0:32],
                        lhsT=w2_sb[:, hk, ut * 128:(ut + 1) * 128],
                        rhs=hidT[:, hk, :],
                        start=(hk == 0), stop=False)
                nc.tensor.matmul(
                    php[:, 0:32],
                    lhsT=bsum[0:1, ut * 128:(ut + 1) * 128],
                    rhs=ones32_r[0:1, :].opt(),
                    start=False, stop=True)
                nc.vector.tensor_copy(projhT[:, ut * BS:(ut + 1) * BS], php[:, 0:32])

        # ---- main loop over batch pairs ----
        flat_out = ctx_out.rearrange("b d -> (b d)")
        out_stage = None
        for pi in range(BS // 2):
            if pi % 2 == 0:
                out_stage = outst_pool.tile([1, 4 * D], F32, tag="out_stage")
            fnats = []
            for half in range(2):
                b = 2 * pi + half
                fnatA = fnat_pool.tile([128, D], BF16, tag=f"fnatA{half}")
                fnatB = fnat_pool.tile([128, D], BF16, tag=f"fnatB{half}")
                nc.gpsimd.dma_start(fnatA[:, :], feat[b, 0:128, :])
                nc.gpsimd.dma_start(fnatB[0:L1, :], feat[b, 128:L, :])
                fnats.append((fnatA, fnatB))

            # transpose features of both halves -> featT [d_p, k, pairL]
            # One psum tile [128, 452] per k holds all 4 transposed chunks
            # (halves x A/B); a single DVE copy moves both halves. The B
            # chunks transpose 98 rows (97 valid; garbage col excluded by
            # the 0:225 slice of each 226-wide group).
            featT = featT_pool.tile([128, DK, 2 * L], BF16, tag="featT")
            for k in range(DK):
                trp = pp.tile([128, 452], BF16, tag="trp", bufs=3)
                for half in range(2):
                    fnatA, fnatB = fnats[half]
                    o = 226 * half
                    nc.tensor.transpose(
                        trp[:, o:o + 128],
                        fnatA[:, k * 128:(k + 1) * 128],
                        ident_r[:, :])
                    nc.tensor.transpose(
                        trp[:, o + 128:o + 128 + L1 + 1],
                        fnatB[0:L1 + 1, k * 128:(k + 1) * 128],
                        ident_r[0:L1 + 1, 0:L1 + 1])
                nc.vector.tensor_copy(
                    featT[:, k, :],
                    trp.rearrange("p (h x) -> p h x", h=2)[:, :, 0:L])

            # scoreT = tanh(W1.T @ featT + projh_bias)  [u, pairL]
            score_sb = score_pool.tile([128, 2, 2 * L], BF16, tag="score_sb")
            for ut in range(2):
                scp = pp.tile([128, 512], F32, tag=f"scp{ut}", bufs=1)
                for k in range(DK):
                    nc.tensor.matmul(
                        scp[:, 0:2 * L],
                        lhsT=w1_sb[:, k, ut * 128:(ut + 1) * 128],
                        rhs=featT[:, k, :],
                        start=(k == 0), stop=(k == DK - 1))
                for half in range(2):
                    b = 2 * pi + half
                    nc.scalar.activation(
                        score_sb[:, ut, half * L:(half + 1) * L],
                        scp[:, half * L:(half + 1) * L],
                        AF.Tanh,
                        bias=projhT[:, ut * BS + b:ut * BS + b + 1])

            # logits [1, 2L] = V.T @ scoreT
            lgp = pp.tile([1, 512], F32, tag="lgp", bufs=1)
            for ut in range(2):
                nc.tensor.matmul(
                    lgp[0:1, 0:2 * L],
                    lhsT=v_sb[:, ut:ut + 1],
                    rhs=score_sb[:, ut, :],
                    start=(ut == 0), stop=(ut == 1))

            # exp + per-half sums; attn = expl / sum
            expl = small.tile([1, 2 * L], F32, tag="expl")
            esum = small.tile([1, 2], F32, tag="esum")
            for half in range(2):
                nc.scalar.activation(
                    expl[0:1, half * L:(half + 1) * L],
                    lgp[0:1, half * L:(half + 1) * L],
                    AF.Exp,
                    accum_out=esum[0:1, half:half + 1])
            rsum = small.tile([1, 2], F32, tag="rsum")
            nc.vector.reciprocal(rsum[:, :], esum[:, :])

            # attnT columns: [l_p, half*2+chunk]
            atp = pp.tile([128, 256], F32, tag="trp", bufs=3)
            attnT = small.tile([128, 4], BF16, tag="attnT")
            for half in range(2):
                nc.tensor.transpose(
                    atp[0:128, 2 * half:2 * half + 1],
                    expl[0:1, half * L:half * L + 128],
                    ident[0:1, 0:1])
                nc.tensor.transpose(
                    atp[0:L1, 2 * half + 1:2 * half + 2],
                    expl[0:1, half * L + 128:half * L + L],
                    ident[0:1, 0:1])
                nc.vector.tensor_copy(attnT[0:128, 2 * half:2 * half + 1],
                                      atp[0:128, 2 * half:2 * half + 1])
                nc.vector.tensor_copy(attnT[0:L1, 2 * half + 1:2 * half + 2],
                                      atp[0:L1, 2 * half + 1:2 * half + 2])

            # context[b, :] = attnT.T @ feat_natural
            for half in range(2):
                b = 2 * pi + half
                fnatA, fnatB = fnats[half]
                for doff, dw in ((0, 512), (512, 512), (1024, 256)):
                    cxp = pp.tile([1, 512], F32, tag="cxp", bufs=2)
                    nc.tensor.matmul(
                        cxp[0:1, 0:dw],
                        lhsT=attnT[0:128, 2 * half:2 * half + 1],
                        rhs=fnatA[:, doff:doff + dw],
                        start=True, stop=False)
                    nc.tensor.matmul(
                        cxp[0:1, 0:dw],
                        lhsT=attnT[0:L1, 2 * half + 1:2 * half + 2],
                        rhs=fnatB[0:L1, doff:doff + dw],
                        start=False, stop=True)
                    nc.scalar.mul(
                        out_stage[0:1, (b % 4) * D + doff:(b % 4) * D + doff + dw],
                        cxp[0:1, 0:dw],
                        rsum[0:1, half:half + 1])

            if pi % 2 == 1:
                g = pi // 2
                nc.sync.dma_start(
                    flat_out[4 * g * D:(4 * g + 4) * D][None, :],
                    out_stage[0:1, :])


_CACHE = {}


def _get_nc():
    if "nc" not in _CACHE:
        _CACHE["nc"] = build_kernel()
    return _CACHE["nc"]


def _run(inputs, trace=False):
    nc = _get_nc()
    in_maps = []
    for c in range(NCORES):
        sl = slice(c * BS, (c + 1) * BS)
        in_maps.append({
            "features": np.ascontiguousarray(inputs["features"][sl]),
            "hidden": np.ascontiguousarray(inputs["hidden"][sl]),
            "W1": np.ascontiguousarray(inputs["W1"]),
            "b1": np.ascontiguousarray(inputs["b1"]),
            "W2": np.ascontiguousarray(inputs["W2"]),
            "b2": np.ascontiguousarray(inputs["b2"]),
            "V": np.ascontiguousarray(inputs["V"]),
            "bV": np.ascontiguousarray(inputs["bV"]),
        })
    res = run_bass_kernel_spmd(nc, in_maps, core_ids=list(range(NCORES)),
                               trace=trace)
    out = np.concatenate([rr["context"] for rr in res.results], axis=0)
    return out, res


def kernel(**inputs):
    out, _ = _run(inputs, trace=False)
    return out


# revision 11
# speedup vs baseline: 2.9860x; 1.0380x over previous
"""Bahdanau attention kernel for Trainium2, 8-core data-parallel.

Shapes (hardcoded): features [256,225,1280] f32, hidden [256,256] f32,
W1 [1280,256], b1 [256], W2 [256,256], b2 [256], V [256,1], bV [1].
Output: context [256,1280] f32.

Sharding: batch dim split across 8 cores (32 per core); parameters
replicated. No collectives.

Per-core algorithm (batch shard of 32, processed in pairs):
  - load features[b] natural layout [L,D] (2 L-chunks of <=128 partitions)
  - PE-transpose 128x128 blocks -> featT [D,L] (fp32r, 1.5 cyc/row)
  - scoreT[u,l] = tanh(W1.T @ featT + (W2.T @ hiddenT + b1 + b2)) with the
    proj_h+bias term applied as the per-partition bias of the ScalarE tanh
  - logits = V.T @ scoreT  (batch-pair moving dim 450 >= 256 so fp32r
    matmuls run at 1 cycle/row)
  - attn = exp(logits) / sum(exp(logits))  (no max-subtraction needed:
    |logits| <= sum|V| so exp is safe in fp32; bV dropped: softmax-invariant)
  - context = attnT.T @ features_natural  (fp32r, N=512 chunks)
"""

import numpy as np

import concourse.bass as bass
import concourse.bacc as bacc
import concourse.tile as tile
import concourse.mybir as mybir
from concourse import masks
from concourse.bass_utils import run_bass_kernel_spmd

B, L, D, H, U = 256, 225, 1280, 256, 256
NCORES = 8
BS = B // NCORES          # 32 batch items per core
L0, L1 = 128, L - 128     # 128 + 97
DK = D // 128             # 10 d-tiles
F32 = mybir.dt.float32
F32R = mybir.dt.float32r
BF16 = mybir.dt.bfloat16
AF = mybir.ActivationFunctionType


def r(ap):
    """View an fp32 AP as float32r (same bits, fast PE path)."""
    return ap.bitcast(F32R)


def build_kernel():
    nc = bacc.Bacc("TRN2", target_bir_lowering=False, debug=False, num_devices=NCORES)

    feat = nc.dram_tensor("features", [BS, L, D], F32, kind="ExternalInput").ap()
    hid = nc.dram_tensor("hidden", [BS, H], F32, kind="ExternalInput").ap()
    w1 = nc.dram_tensor("W1", [D, U], F32, kind="ExternalInput").ap()
    b1 = nc.dram_tensor("b1", [U], F32, kind="ExternalInput").ap()
    w2 = nc.dram_tensor("W2", [H, U], F32, kind="ExternalInput").ap()
    b2 = nc.dram_tensor("b2", [U], F32, kind="ExternalInput").ap()
    v = nc.dram_tensor("V", [U, 1], F32, kind="ExternalInput").ap()
    nc.dram_tensor("bV", [1], F32, kind="ExternalInput")  # softmax-invariant
    ctx_out = nc.dram_tensor("context", [BS, D], F32, kind="ExternalOutput").ap()

    with tile.TileContext(nc) as tc:
        body(tc, feat, hid, w1, b1, w2, b2, v, ctx_out)
    nc.compile()
    return nc


def body(tc, feat, hid, w1, b1, w2, b2, v, ctx_out):
    nc = tc.nc
    from contextlib import ExitStack

    with ExitStack() as ctx:
        const = ctx.enter_context(tc.tile_pool(name="const", bufs=1))
        fnat_pool = ctx.enter_context(tc.tile_pool(name="fnat", bufs=6))
        featT_pool = ctx.enter_context(tc.tile_pool(name="featT", bufs=3))
        score_pool = ctx.enter_context(tc.tile_pool(name="score", bufs=3))
        small = ctx.enter_context(tc.tile_pool(name="small", bufs=2))
        outst_pool = ctx.enter_context(tc.tile_pool(name="outst", bufs=2))
        # One PSUM pool; bank budget (8 total): trp*2 + scp0 + scp1 + lgp + cxp*2 = 7
        pp = ctx.enter_context(tc.tile_pool(name="pp", bufs=1, space="PSUM"))

        # ---- constants ----
        ident = const.tile([128, 128], F32)
        masks.make_identity(nc, ident[:, :])
        ident_r = const.tile([128, 128], BF16)
        nc.vector.tensor_copy(ident_r[:, :], ident[:, :])

        ones32 = const.tile([1, 32], F32)
        nc.gpsimd.memset(ones32[:, :], 1.0)
        ones32_r = const.tile([1, 32], BF16)
        nc.vector.tensor_copy(ones32_r[:, :], ones32[:, :])

        w1_sb = const.tile([128, DK, U], BF16)  # [d_in_tile, d_tile, u]
        nc.gpsimd.dma_start(w1_sb[:, :, :], w1.rearrange("(k p) u -> p k u", p=128))

        w2_sb = const.tile([128, 2, U], BF16)   # [h_in_tile, h_tile, u]
        nc.gpsimd.dma_start(w2_sb[:, :, :], w2.rearrange("(k p) u -> p k u", p=128))

        v_sb = const.tile([128, 2], BF16)       # [u_in_tile, u_tile]
        nc.gpsimd.dma_start(v_sb[:, :], v.rearrange("(t p) o -> p (t o)", p=128))

        bsum = const.tile([1, U], BF16)         # b1 + b2 (both added pre-tanh)
        b1_sb = const.tile([1, U], F32)
        b2_sb = const.tile([1, U], F32)
        nc.sync.dma_start(b1_sb[:, :], b1[None, :])
        nc.sync.dma_start(b2_sb[:, :], b2[None, :])
        nc.vector.tensor_add(bsum[:, :], b1_sb[:, :], b2_sb[:, :])

        # ---- proj_hT [u, b] = W2.T @ hiddenT + (b1+b2) ----
        hid_nat = const.tile([32, H], F32)
        nc.sync.dma_start(hid_nat[:, :], hid[:, :])

        hidT = const.tile([128, 2, BS], BF16)   # [h_in_tile, h_tile, b]
        if True:
            for hk in range(2):
                hp = pp.tile([128, 256], F32, tag="trp", bufs=2)
                nc.tensor.transpose(
                    hp[:, 0:32], hid_nat[0:32, hk * 128:(hk + 1) * 128],
                    ident[0:32, 0:32])
                nc.vector.tensor_copy(hidT[:, hk, :], hp[:, 0:32])
            projhT = const.tile([128, 2 * BS], F32)  # [u_in_tile, ut*32+b]
            for ut in range(2):
                php = pp.tile([128, 256], F32, tag="trp", bufs=2)
                for hk in range(2):
                    nc.tensor.matmul(
                        php[:, 0:32],
                        lhsT=w2_sb[:, hk, ut * 128:(ut + 1) * 128],
                        rhs=hidT[:, hk, :],
                        start=(hk == 0), stop=False)
                nc.tensor.matmul(
                    php[:, 0:32],
                    lhsT=bsum[0:1, ut * 128:(ut + 1) * 128],
                    rhs=ones32_r[0:1, :].opt(),
                    start=False, stop=True)
                nc.vector.tensor_copy(projhT[:, ut * BS:(ut + 1) * BS], php[:, 0:32])

        # ---- main loop over batch pairs, software-pipelined ----
        # head(p): loads, feature transposes, step-1 matmuls, tanh
        # tail(p): logits, attn transposes, context  (emitted after
        # head(p+1) so the in-order PE stream always has independent
        # transpose work while ScalarE finishes tanh/exp -> no PE gaps,
        # HAM stays at full clock)
        flat_out = ctx_out.rearrange("b d -> (b d)")
        NPAIR = BS // 2
        st = {}

        def head(pi):
            fnats = []
            for half in range(2):
                b = 2 * pi + half
                fnatA = fnat_pool.tile([128, D], BF16, tag=f"fnatA{half}",
                                       name=f"fnatA_{pi}_{half}")
                fnatB = fnat_pool.tile([128, D], BF16, tag=f"fnatB{half}",
                                       name=f"fnatB_{pi}_{half}")
                nc.gpsimd.dma_start(fnatA[:, :], feat[b, 0:128, :])
                nc.gpsimd.dma_start(fnatB[0:L1, :], feat[b, 128:L, :])
                fnats.append((fnatA, fnatB))
            st[("fnats", pi)] = fnats

            featT = featT_pool.tile([128, DK, 2 * L], BF16, tag="featT",
                                    name=f"featT_{pi}")
            for k in range(DK):
                trp = pp.tile([128, 452], BF16, tag="trp", bufs=2,
                              name=f"trp_{pi}_{k}")
                for half in range(2):
                    fnatA, fnatB = fnats[half]
                    o = 226 * half
                    nc.tensor.transpose(
                        trp[:, o:o + 128],
                        fnatA[:, k * 128:(k + 1) * 128],
                        ident_r[:, :])
                    nc.tensor.transpose(
                        trp[:, o + 128:o + 128 + L1 + 1],
                        fnatB[0:L1 + 1, k * 128:(k + 1) * 128],
                        ident_r[0:L1 + 1, 0:L1 + 1])
                nc.vector.tensor_copy(
                    featT[:, k, :],
                    trp.rearrange("p (h x) -> p h x", h=2)[:, :, 0:L])

            score_sb = score_pool.tile([128, 2, 2 * L], BF16, tag="score_sb",
                                       name=f"score_{pi}")
            for ut in range(2):
                scp = pp.tile([128, 512], F32, tag=f"scp{ut}", bufs=1,
                              name=f"scp_{pi}_{ut}")
                for k in range(DK):
                    nc.tensor.matmul(
                        scp[:, 0:2 * L],
                        lhsT=w1_sb[:, k, ut * 128:(ut + 1) * 128],
                        rhs=featT[:, k, :],
                        start=(k == 0), stop=(k == DK - 1))
                for half in range(2):
                    b = 2 * pi + half
                    nc.scalar.activation(
                        score_sb[:, ut, half * L:(half + 1) * L],
                        scp[:, half * L:(half + 1) * L],
                        AF.Tanh,
                        bias=projhT[:, ut * BS + b:ut * BS + b + 1])
            st[("score", pi)] = score_sb

        def tail(pi):
            score_sb = st.pop(("score", pi))
            fnats = st.pop(("fnats", pi))
            if pi % 2 == 0:
                st["out_stage"] = outst_pool.tile([1, 4 * D], F32,
                                                  tag="out_stage",
                                                  name=f"outst_{pi // 2}")
            out_stage = st["out_stage"]

            lgp = pp.tile([128, 512], F32, tag="lgp", bufs=2,
                          name=f"lgp_{pi}")
            for ut in range(2):
                nc.tensor.matmul(
                    lgp[0:1, 0:2 * L],
                    lhsT=v_sb[:, ut:ut + 1],
                    rhs=score_sb[:, ut, :],
                    start=(ut == 0), stop=(ut == 1))

            expl = small.tile([1, 2 * L], F32, tag="expl", name=f"expl_{pi}")
            esum = small.tile([1, 2], F32, tag="esum", name=f"esum_{pi}")
            for half in range(2):
                nc.scalar.activation(
                    expl[0:1, half * L:(half + 1) * L],
                    lgp[0:1, half * L:(half + 1) * L],
                    AF.Exp,
                    accum_out=esum[0:1, half:half + 1])
            rsum = small.tile([1, 2], F32, tag="rsum", name=f"rsum_{pi}")
            nc.vector.reciprocal(rsum[:, :], esum[:, :])

            atp = pp.tile([128, 512], F32, tag="lgp", bufs=2,
                          name=f"atp_{pi}")
            attnT = small.tile([128, 4], BF16, tag="attnT", name=f"attnT_{pi}")
            for half in range(2):
                nc.tensor.transpose(
                    atp[0:128, 2 * half:2 * half + 1],
                    expl[0:1, half * L:half * L + 128],
                    ident[0:1, 0:1])
                nc.tensor.transpose(
                    atp[0:L1, 2 * half + 1:2 * half + 2],
                    expl[0:1, half * L + 128:half * L + L],
                    ident[0:1, 0:1])
                nc.vector.tensor_copy(attnT[0:128, 2 * half:2 * half + 1],
                                      atp[0:128, 2 * half:2 * half + 1])
                nc.vector.tensor_copy(attnT[0:L1, 2 * half + 1:2 * half + 2],
                                      atp[0:L1, 2 * half + 1:2 * half + 2])

            for half in range(2):
                b = 2 * pi + half
                fnatA, fnatB = fnats[half]
                for doff, dw in ((0, 512), (512, 512), (1024, 256)):
                    cxp = pp.tile([1, 512], F32, tag="cxp", bufs=2,
                                  name=f"cxp_{pi}_{half}_{doff}")
                    nc.tensor.matmul(
                        cxp[0:1, 0:dw],
                        lhsT=attnT[0:128, 2 * half:2 * half + 1],
                        rhs=fnatA[:, doff:doff + dw],
                        start=True, stop=False)
                    nc.tensor.matmul(
                        cxp[0:1, 0:dw],
                        lhsT=attnT[0:L1, 2 * half + 1:2 * half + 2],
                        rhs=fnatB[0:L1, doff:doff + dw],
                        start=False, stop=True)
                    nc.scalar.mul(
                        out_stage[0:1, (b % 4) * D + doff:(b % 4) * D + doff + dw],
                        cxp[0:1, 0:dw],
                        rsum[0:1, half:half + 1])

            if pi % 2 == 1:
                g = pi // 2
                nc.sync.dma_start(
                    flat_out[4 * g * D:(4 * g + 4) * D][None, :],
                    out_stage[0:1, :])

        for pi in range(NPAIR):
            head(pi)
            if pi >= 1:
                tail(pi - 1)
        tail(NPAIR - 1)


def _enable_jax_cache():
    try:
        import jax
        jax.config.update("jax_compilation_cache_dir", "/tmp/jax_neff_cache")
        jax.config.update("jax_persistent_cache_min_entry_size_bytes", 0)
        jax.config.update("jax_persistent_cache_min_compile_time_secs", 0)
    except Exception:
        pass


_enable_jax_cache()

_CACHE = {}


def _get_nc():
    if "nc" not in _CACHE:
        _CACHE["nc"] = build_kernel()
    return _CACHE["nc"]


def _run(inputs, trace=False):
    nc = _get_nc()
    in_maps = []
    for c in range(NCORES):
        sl = slice(c * BS, (c + 1) * BS)
        in_maps.append({
            "features": np.ascontiguousarray(inputs["features"][sl]),
            "hidden": np.ascontiguousarray(inputs["hidden"][sl]),
            "W1": np.ascontiguousarray(inputs["W1"]),
            "b1": np.ascontiguousarray(inputs["b1"]),
            "W2": np.ascontiguousarray(inputs["W2"]),
            "b2": np.ascontiguousarray(inputs["b2"]),
            "V": np.ascontiguousarray(inputs["V"]),
            "bV": np.ascontiguousarray(inputs["bV"]),
        })
    res = run_bass_kernel_spmd(nc, in_maps, core_ids=list(range(NCORES)),
                               trace=trace)
    out = np.concatenate([rr["context"] for rr in res.results], axis=0)
    return out, res


def kernel(**inputs):
    out, _ = _run(inputs, trace=False)
    return out
